# revision 9
# baseline (speedup 1.0000x reference)
"""Multi-head attention (B=4, S=2048, D=1024, H=16, DK=64) on 8 Trainium2
NeuronCores.

Sharding: core c = 2*b + j handles batch b = c//2 and query rows
[j*1024, (j+1)*1024).  Each core recomputes full-S K/V for its batch
(fully local, no collectives); outputs concatenate.

Schedule (designed so the Scalar engine's exp stream — the critical
resource at ~294us/core — starts as early as possible):
  1. V = X@Wv for all 16 s-chunks (xt+wv resident, freed afterwards)
  2. per head-pair i: K^T[dh=i] (xt re-streamed from DRAM),
     Q^T[pair i] (wq streamed), then heads 2i, 2i+1:
     scores^T -> exp (ACT, mask+(-3) bias folded in) -> PV with a ones
     column producing the softmax denominator in row 64
  3. normalize via reciprocal + partition_broadcast + DVE mul
  4. out = ctx^T-as-lhsT @ Wo + radd, radd = bv@Wo + bo folded on host

All matmuls bf16 (1 cycle/row), fp32 PSUM accumulate, fp32 output.
"""

import numpy as np
import ml_dtypes

B, S, D, H, DK = 4, 2048, 1024, 16, 64
SQ = S // 2          # query rows per core
N_CORES = 8
SH = DK + 1          # per-head V width incl. ones column
NEG_C = -3.0         # exp stabilizer; cancels exactly in normalization
BF = ml_dtypes.bfloat16


def _build():
    import concourse.mybir as mybir
    import concourse.tile as tile
    from concourse import bacc

    dt = mybir.dt
    AF = mybir.ActivationFunctionType
    nc = bacc.Bacc("TRN2", num_devices=N_CORES)

    xt = nc.declare_dram_parameter("xt", [D, S], dt.bfloat16, isOutput=False)
    xqt = nc.declare_dram_parameter("xqt", [D, SQ], dt.bfloat16, isOutput=False)
    wq = nc.declare_dram_parameter("wq", [D, D], dt.bfloat16, isOutput=False)
    wk = nc.declare_dram_parameter("wk", [D, D], dt.bfloat16, isOutput=False)
    wv = nc.declare_dram_parameter("wv", [D, D], dt.bfloat16, isOutput=False)
    wo = nc.declare_dram_parameter("wo", [D, D], dt.bfloat16, isOutput=False)
    bq = nc.declare_dram_parameter("bq", [D], dt.float32, isOutput=False)
    bk = nc.declare_dram_parameter("bk", [D], dt.float32, isOutput=False)
    radd = nc.declare_dram_parameter("radd", [D], dt.float32, isOutput=False)
    mk = nc.declare_dram_parameter("mk", [S], dt.float32, isOutput=False)
    out = nc.declare_dram_parameter("out", [SQ, D], dt.float32, isOutput=True)

    with tile.TileContext(nc) as tc:
        with (
            tc.tile_pool(name="pers", bufs=1) as pers,
            tc.tile_pool(name="ps", bufs=3, space="PSUM") as ps,
            tc.tile_pool(name="ctxp", bufs=2, space="PSUM") as ctxp,
        ):
            # ---- persistent SBUF arrays -------------------------------
            qt_s = pers.tile([128, 8 * SQ], dt.bfloat16, tag="qt")
            kt_s = pers.tile([128, 8 * S], dt.bfloat16, tag="kt")
            v_s = pers.tile([128, 16 * H * SH], dt.bfloat16, tag="v")
            ctxt_s = pers.tile([128, 8 * SQ], dt.bfloat16, tag="ctxt")
            bqc = pers.tile([128, 8], dt.float32, tag="bqc")
            bkc = pers.tile([128, 8], dt.float32, tag="bkc")
            mkc = pers.tile([128, 16], dt.float32, tag="mkc")

            nc.sync.dma_start(out=bqc, in_=bq.rearrange("(a p) -> p a", p=128))
            nc.sync.dma_start(out=bkc, in_=bk.rearrange("(a p) -> p a", p=128))
            nc.sync.dma_start(out=mkc, in_=mk.rearrange("(a p) -> p a", p=128))

            with (
                tc.tile_pool(name="poolL", bufs=1) as poolL,
                tc.tile_pool(name="xts", bufs=3) as xts,
                tc.tile_pool(name="wqs", bufs=2) as wqs,
            ):
                xqt_s = poolL.tile([128, 8 * SQ], dt.bfloat16, tag="xqt")
                wk_s = poolL.tile([128, 8 * D], dt.bfloat16, tag="wk")

                # ---- phase 1: V (xt + wv resident, freed after) -------
                with tc.tile_pool(name="poolA", bufs=1) as poolA:
                    xt_s = poolA.tile([128, 8 * S], dt.bfloat16, tag="xt")
                    wv_s = poolA.tile([128, 8 * D], dt.bfloat16, tag="wv")
                    for c in range(8):
                        nc.sync.dma_start(
                            out=xt_s[:, c * S:(c + 1) * S],
                            in_=xt[c * 128:(c + 1) * 128, :])
                        nc.sync.dma_start(
                            out=wv_s[:, c * D:(c + 1) * D],
                            in_=wv[c * 128:(c + 1) * 128, :])
                    for c in range(8):
                        nc.sync.dma_start(
                            out=wk_s[:, c * D:(c + 1) * D],
                            in_=wk[c * 128:(c + 1) * 128, :])
                        nc.sync.dma_start(
                            out=xqt_s[:, c * SQ:(c + 1) * SQ],
                            in_=xqt[c * 128:(c + 1) * 128, :])

                    for sc in range(16):
                        pv = ps.tile([128, 1024], dt.float32, tag="ps",
                                     name=f"pv{sc}")
                        for c in range(8):
                            lhsT = xt_s[:, c * S + sc * 128: c * S + (sc + 1) * 128]
                            for dv2 in range(2):
                                nc.tensor.matmul(
                                    out=pv[:, dv2 * 512:(dv2 + 1) * 512],
                                    lhsT=lhsT,
                                    rhs=wv_s[:, c * D + dv2 * 512: c * D + (dv2 + 1) * 512],
                                    start=(c == 0), stop=(c == 7))
                        v3 = v_s[:, sc * H * SH:(sc + 1) * H * SH].rearrange(
                            "p (h e) -> p h e", e=SH)
                        nc.gpsimd.memset(v3[:, :, DK:SH], 1.0)
                        for dv2 in range(2):
                            nc.vector.tensor_copy(
                                v3[:, dv2 * 8:(dv2 + 1) * 8, 0:DK],
                                pv[:, dv2 * 512:(dv2 + 1) * 512].rearrange(
                                    "p (h d) -> p h d", d=DK))

                # ---- phase 2: per-pair K^T, Q^T, attention ------------
                with (
                    tc.tile_pool(name="attin", bufs=1) as attin,
                    tc.tile_pool(name="epool", bufs=4) as epool,
                    tc.tile_pool(name="rpool", bufs=1) as rpool,
                    tc.tile_pool(name="opool", bufs=2) as opool,
                ):
                    wo_s = attin.tile([128, 8 * D], dt.bfloat16, tag="wo")
                    bob = attin.tile([128, D], dt.float32, tag="bob")
                    for c in range(8):
                        nc.sync.dma_start(
                            out=wo_s[:, c * D:(c + 1) * D],
                            in_=wo[c * 128:(c + 1) * 128, :])

                    def _bcast_src(ap):
                        import concourse.bass as bass
                        return bass.AP(
                            tensor=ap.tensor, offset=ap.offset,
                            ap=[[0, 128]] + [list(p) for p in ap.ap])

                    nc.gpsimd.dma_start(out=bob, in_=_bcast_src(radd[:]))

                    for i in range(8):
                        # K^T chunk dh=i, xt streamed from DRAM per half
                        for hf in range(2):
                            pk = ps.tile([128, 1024], dt.float32, tag="ps",
                                         name=f"pk{i}_{hf}")
                            for c in range(8):
                                xtc = xts.tile([128, 1024], dt.bfloat16,
                                               tag="xts", name=f"xtc{i}_{hf}_{c}")
                                nc.sync.dma_start(
                                    out=xtc,
                                    in_=xt[c * 128:(c + 1) * 128,
                                           hf * 1024:(hf + 1) * 1024])
                                lhsT = wk_s[:, c * D + i * 128: c * D + (i + 1) * 128]
                                for st in range(2):
                                    nc.tensor.matmul(
                                        out=pk[:, st * 512:(st + 1) * 512],
                                        lhsT=lhsT,
                                        rhs=xtc[:, st * 512:(st + 1) * 512],
                                        start=(c == 0), stop=(c == 7))
                            nc.vector.tensor_scalar_add(
                                kt_s[:, i * S + hf * 1024: i * S + (hf + 1) * 1024],
                                pk, bkc[:, i:i + 1])

                        # Q^T pair i (wq streamed from DRAM)
                        wqc = wqs.tile([128, 1024], dt.bfloat16, tag="wqs",
                                       name=f"wqc{i}")
                        nc.sync.dma_start(
                            out=wqc.rearrange("p (c n) -> p c n", n=128),
                            in_=wq.rearrange("(c p) n -> p c n", p=128)[
                                :, :, i * 128:(i + 1) * 128])
                        pq = ps.tile([128, 1024], dt.float32, tag="ps",
                                     name=f"pq{i}")
                        for c in range(8):
                            lhsT = wqc[:, c * 128:(c + 1) * 128]
                            for q2 in range(2):
                                nc.tensor.matmul(
                                    out=pq[:, q2 * 512:(q2 + 1) * 512],
                                    lhsT=lhsT,
                                    rhs=xqt_s[:, c * SQ + q2 * 512: c * SQ + (q2 + 1) * 512],
                                    start=(c == 0), stop=(c == 7))
                        nc.vector.tensor_scalar_add(
                            qt_s[:, i * SQ:(i + 1) * SQ], pq, bqc[:, i:i + 1])

                        for h in (2 * i, 2 * i + 1):
                            po = (h % 2) * 64
                            cx = [ctxp.tile([SH, 512], dt.float32, tag="cx",
                                            name=f"cx{h}_{q2}") for q2 in range(2)]
                            for sc in range(16):
                                sp = ps.tile([128, 1024], dt.float32, tag="ps",
                                             name=f"sp{h}_{sc}")
                                lhsT = kt_s[po:po + 64,
                                            i * S + sc * 128: i * S + (sc + 1) * 128]
                                for q2 in range(2):
                                    nc.tensor.matmul(
                                        out=sp[:, q2 * 512:(q2 + 1) * 512],
                                        lhsT=lhsT,
                                        rhs=qt_s[po:po + 64,
                                                 i * SQ + q2 * 512: i * SQ + (q2 + 1) * 512],
                                        start=True, stop=True)
                                e = epool.tile([128, 1024], dt.bfloat16,
                                               tag="e", name=f"e{h}_{sc}")
                                nc.scalar.activation(
                                    out=e, in_=sp, func=AF.Exp,
                                    bias=mkc[:, sc:sc + 1], scale=1.0 / np.sqrt(DK))
                                vh = v_s[:, sc * H * SH + h * SH:
                                         sc * H * SH + (h + 1) * SH]
                                for q2 in range(2):
                                    nc.tensor.matmul(
                                        out=cx[q2], lhsT=vh,
                                        rhs=e[:, q2 * 512:(q2 + 1) * 512],
                                        start=(sc == 0), stop=(sc == 15))
                            rcp = rpool.tile([1, 1024], dt.float32, tag="rcp",
                                             name=f"rcp{h}")
                            for q2 in range(2):
                                nc.vector.reciprocal(
                                    out=rcp[:, q2 * 512:(q2 + 1) * 512],
                                    in_=cx[q2][DK:SH, :])
                            rb = rpool.tile([64, 1024], dt.float32, tag="rb",
                                            name=f"rb{h}")
                            nc.gpsimd.partition_broadcast(rb, rcp[0:1, :])
                            for q2 in range(2):
                                nc.vector.tensor_mul(
                                    out=ctxt_s[po:po + 64,
                                               i * SQ + q2 * 512: i * SQ + (q2 + 1) * 512],
                                    in0=cx[q2][0:DK, :],
                                    in1=rb[:, q2 * 512:(q2 + 1) * 512])

                    # ---- phase 3: output projection -------------------
                    for qc in range(8):
                        pO = ps.tile([128, 1024], dt.float32, tag="ps",
                                     name=f"pO{qc}")
                        for i in range(8):
                            lhsT = ctxt_s[:, i * SQ + qc * 128: i * SQ + (qc + 1) * 128]
                            for do2 in range(2):
                                nc.tensor.matmul(
                                    out=pO[:, do2 * 512:(do2 + 1) * 512],
                                    lhsT=lhsT,
                                    rhs=wo_s[:, i * D + do2 * 512: i * D + (do2 + 1) * 512],
                                    start=(i == 0), stop=(i == 7))
                        ot = opool.tile([128, 1024], dt.float32, tag="ot",
                                        name=f"ot{qc}")
                        nc.vector.tensor_add(out=ot, in0=pO, in1=bob)
                        nc.sync.dma_start(
                            out=out[qc * 128:(qc + 1) * 128, :], in_=ot)

    nc.compile()
    return nc


def _make_in_maps(inputs):
    hidden_states = inputs["hidden_states"]
    attention_mask = inputs["attention_mask"]
    wq_b = np.ascontiguousarray(np.asarray(inputs["Wq"]).astype(BF))
    wk_b = np.ascontiguousarray(np.asarray(inputs["Wk"]).astype(BF))
    wv_b = np.ascontiguousarray(np.asarray(inputs["Wv"]).astype(BF))
    wo_b = np.ascontiguousarray(np.asarray(inputs["Wo"]).astype(BF))
    bq_f = np.ascontiguousarray(np.asarray(inputs["bq"]).astype(np.float32))
    bk_f = np.ascontiguousarray(np.asarray(inputs["bk"]).astype(np.float32))
    radd = (np.asarray(inputs["bv"]).astype(np.float32) @
            np.asarray(inputs["Wo"]).astype(np.float32) +
            np.asarray(inputs["bo"]).astype(np.float32))
    radd = np.ascontiguousarray(radd.astype(np.float32))

    in_maps = []
    for c in range(N_CORES):
        b, j = c // 2, c % 2
        xt_b = np.ascontiguousarray(np.asarray(hidden_states[b]).T.astype(BF))
        in_maps.append({
            "xt": xt_b,
            "xqt": np.ascontiguousarray(xt_b[:, j * SQ:(j + 1) * SQ]),
            "wq": wq_b, "wk": wk_b, "wv": wv_b, "wo": wo_b,
            "bq": bq_f, "bk": bk_f, "radd": radd,
            "mk": np.ascontiguousarray(
                np.asarray(attention_mask[b, 0, 0, :]).astype(np.float32) + NEG_C),
        })
    return in_maps


def kernel(hidden_states, attention_mask, Wq, bq, Wk, bk, Wv, bv, Wo, bo):
    from concourse.bass_utils import run_bass_kernel_spmd

    nc = _build()
    in_maps = _make_in_maps(dict(
        hidden_states=hidden_states, attention_mask=attention_mask,
        Wq=Wq, bq=bq, Wk=Wk, bk=bk, Wv=Wv, bv=bv, Wo=Wo, bo=bo))
    res = run_bass_kernel_spmd(nc, in_maps, list(range(N_CORES)))

    full = np.empty((B, S, D), dtype=np.float32)
    for c in range(N_CORES):
        b, j = c // 2, c % 2
        full[b, j * SQ:(j + 1) * SQ, :] = res.results[c]["out"]
    return full


# revision 11
# speedup vs baseline: 1.3462x; 1.3462x over previous
"""Multi-head attention (B=4, S=2048, D=1024, H=16, DK=64) on 8 Trainium2
NeuronCores.

Sharding: core c = 2*b + j handles batch b = c//2 and query rows
[j*1024, (j+1)*1024).  Each core recomputes full-S K/V for its batch
(fully local, no collectives); outputs concatenate.

Key scheduling idea: the Scalar engine's exp stream (~294us of work)
and the Tensor engine (~383us of matmul work) must both stay busy.
K^T/Q^T projection chains for pair i+1 are statically interleaved as
PE filler between the attention chunks of pair i, so the Tensor engine
never idles long enough for the HAM clock gate to re-throttle it.

Layouts (feature-on-partition for everything left of the softmax):
  X^T [D,S] resident (bf16);  V [S, H*(DK+1)] resident with a ones
  column per head (PV row 64 = softmax denominator);  K^T/Q^T live in
  per-pair streaming tiles;  E^T = exp(scores^T/8 + mask - 3) streamed
  per (head, s-chunk);  ctx^T accumulates; out = ctx^T-as-lhsT @ Wo.
All matmuls bf16 (1 cycle/row), fp32 PSUM, fp32 output.
bv/bo are folded on the host: radd = bv @ Wo + bo (softmax rows sum
to 1, so attn @ (V + bv) @ Wo + bo == attn@V@Wo + radd).
"""

import numpy as np
import ml_dtypes

B, S, D, H, DK = 4, 2048, 1024, 16, 64
SQ = S // 2          # query rows per core
N_CORES = 8
SH = DK + 1          # per-head V width incl. ones column
NEG_C = -3.0         # exp stabilizer; cancels exactly in normalization
BF = ml_dtypes.bfloat16


def _build():
    import concourse.mybir as mybir
    import concourse.tile as tile
    from concourse import bacc

    dt = mybir.dt
    AF = mybir.ActivationFunctionType
    nc = bacc.Bacc("TRN2", num_devices=N_CORES)

    xt = nc.declare_dram_parameter("xt", [D, S], dt.bfloat16, isOutput=False)
    xqt = nc.declare_dram_parameter("xqt", [D, SQ], dt.bfloat16, isOutput=False)
    wq = nc.declare_dram_parameter("wq", [D, D], dt.bfloat16, isOutput=False)
    wk = nc.declare_dram_parameter("wk", [D, D], dt.bfloat16, isOutput=False)
    wv = nc.declare_dram_parameter("wv", [D, D], dt.bfloat16, isOutput=False)
    wo = nc.declare_dram_parameter("wo", [D, D], dt.bfloat16, isOutput=False)
    bq = nc.declare_dram_parameter("bq", [D], dt.float32, isOutput=False)
    bk = nc.declare_dram_parameter("bk", [D], dt.float32, isOutput=False)
    radd = nc.declare_dram_parameter("radd", [D], dt.float32, isOutput=False)
    mk = nc.declare_dram_parameter("mk", [S], dt.float32, isOutput=False)
    out = nc.declare_dram_parameter("out", [SQ, D], dt.float32, isOutput=True)

    with tile.TileContext(nc) as tc:
        with (
            tc.tile_pool(name="pers", bufs=1) as pers,
            tc.tile_pool(name="ps", bufs=2, space="PSUM") as ps,
            tc.tile_pool(name="chain", bufs=1, space="PSUM") as chain,
            tc.tile_pool(name="ctxp", bufs=2, space="PSUM") as ctxp,
            tc.tile_pool(name="ktp", bufs=3) as ktp,
            tc.tile_pool(name="qtp", bufs=3) as qtp,
            tc.tile_pool(name="xstr", bufs=3) as xstr,
            tc.tile_pool(name="wstr", bufs=2) as wstr,
        ):
            # ---- persistent SBUF arrays -------------------------------
            v_s = pers.tile([128, 16 * H * SH], dt.bfloat16, tag="v")
            xt_s = pers.tile([128, 8 * S], dt.bfloat16, tag="xt")
            wk_s = pers.tile([128, 8 * D], dt.bfloat16, tag="wk")
            bqc = pers.tile([128, 8], dt.float32, tag="bqc")
            bkc = pers.tile([128, 8], dt.float32, tag="bkc")
            mkc = pers.tile([128, 16], dt.float32, tag="mkc")

            nc.sync.dma_start(out=bqc, in_=bq.rearrange("(a p) -> p a", p=128))
            nc.sync.dma_start(out=bkc, in_=bk.rearrange("(a p) -> p a", p=128))
            nc.sync.dma_start(out=mkc, in_=mk.rearrange("(a p) -> p a", p=128))

            # ---- phase 1: V (wv freed after) --------------------------
            with tc.tile_pool(name="poolA", bufs=1) as poolA:
                wv_s = poolA.tile([128, 8 * D], dt.bfloat16, tag="wv")
                for c in range(8):
                    nc.sync.dma_start(
                        out=xt_s[:, c * S:(c + 1) * S],
                        in_=xt[c * 128:(c + 1) * 128, :])
                    nc.sync.dma_start(
                        out=wv_s[:, c * D:(c + 1) * D],
                        in_=wv[c * 128:(c + 1) * 128, :])
                for c in range(8):
                    nc.sync.dma_start(
                        out=wk_s[:, c * D:(c + 1) * D],
                        in_=wk[c * 128:(c + 1) * 128, :])

                for sc in range(16):
                    pv = ps.tile([128, 1024], dt.float32, tag="ps",
                                 name=f"pv{sc}")
                    for c in range(8):
                        lhsT = xt_s[:, c * S + sc * 128: c * S + (sc + 1) * 128]
                        for dv2 in range(2):
                            nc.tensor.matmul(
                                out=pv[:, dv2 * 512:(dv2 + 1) * 512],
                                lhsT=lhsT,
                                rhs=wv_s[:, c * D + dv2 * 512: c * D + (dv2 + 1) * 512],
                                start=(c == 0), stop=(c == 7))
                    v3 = v_s[:, sc * H * SH:(sc + 1) * H * SH].rearrange(
                        "p (h e) -> p h e", e=SH)
                    nc.gpsimd.memset(v3[:, :, DK:SH], 1.0)
                    for dv2 in range(2):
                        nc.vector.tensor_copy(
                            v3[:, dv2 * 8:(dv2 + 1) * 8, 0:DK],
                            pv[:, dv2 * 512:(dv2 + 1) * 512].rearrange(
                                "p (h d) -> p h d", d=DK))

            # ---- phase 2: interleaved projections + attention ---------
            with (
                tc.tile_pool(name="attin", bufs=1) as attin,
                tc.tile_pool(name="epool", bufs=6) as epool,
                tc.tile_pool(name="rpool", bufs=1) as rpool,
                tc.tile_pool(name="stg", bufs=2) as stg,
                tc.tile_pool(name="opool", bufs=2) as opool,
            ):
                ctxt_s = attin.tile([128, 8 * SQ], dt.bfloat16, tag="ctxt")
                wo_s = attin.tile([128, 8 * D], dt.bfloat16, tag="wo")
                bob = attin.tile([128, D], dt.float32, tag="bob")
                for c in range(8):
                    nc.sync.dma_start(
                        out=wo_s[:, c * D:(c + 1) * D],
                        in_=wo[c * 128:(c + 1) * 128, :])

                def _bcast_src(ap):
                    import concourse.bass as bass
                    return bass.AP(
                        tensor=ap.tensor, offset=ap.offset,
                        ap=[[0, 128]] + [list(p) for p in ap.ap])

                nc.gpsimd.dma_start(out=bob, in_=_bcast_src(radd[:]))

                kt_tiles = {}
                qt_tiles = {}

                # Filler-unit generators: emit projection chains for pair
                # `i` in small steps so they interleave with attention.
                def k_chain_units(i):
                    kt_t = ktp.tile([128, S], dt.bfloat16, tag="ktt",
                                    name=f"ktt{i}")
                    kt_tiles[i] = kt_t
                    for hf in range(2):
                        pk = chain.tile([128, 1024], dt.float32, tag="chain",
                                        name=f"pk{i}_{hf}")
                        for c in range(8):
                            lhsT = wk_s[:, c * D + i * 128: c * D + (i + 1) * 128]
                            def do_k(c=c, hf=hf, pk=pk, lhsT=lhsT):
                                for st in range(2):
                                    nc.tensor.matmul(
                                        out=pk[:, st * 512:(st + 1) * 512],
                                        lhsT=lhsT,
                                        rhs=xt_s[:, c * S + hf * 1024 + st * 512:
                                                 c * S + hf * 1024 + (st + 1) * 512],
                                        start=(c == 0), stop=(c == 7))
                            yield do_k
                        def drain_k(hf=hf, pk=pk, kt_t=kt_t):
                            nc.vector.tensor_scalar_add(
                                kt_t[:, hf * 1024:(hf + 1) * 1024],
                                pk, bkc[:, i:i + 1])
                        yield drain_k

                def q_chain_units(i):
                    qt_t = qtp.tile([128, SQ], dt.bfloat16, tag="qtt",
                                    name=f"qtt{i}")
                    qt_tiles[i] = qt_t
                    wqc = wstr.tile([128, 1024], dt.bfloat16, tag="wqs",
                                    name=f"wqc{i}")
                    nc.sync.dma_start(
                        out=wqc.rearrange("p (c n) -> p c n", n=128),
                        in_=wq.rearrange("(c p) n -> p c n", p=128)[
                            :, :, i * 128:(i + 1) * 128])
                    pq = chain.tile([128, 1024], dt.float32, tag="chain",
                                    name=f"pq{i}")
                    for c in range(8):
                        xqc = xstr.tile([128, SQ], dt.bfloat16, tag="xqs",
                                        name=f"xqc{i}_{c}")
                        nc.sync.dma_start(
                            out=xqc, in_=xqt[c * 128:(c + 1) * 128, :])
                        def do_q(c=c, pq=pq, wqc=wqc, xqc=xqc):
                            lhsT = wqc[:, c * 128:(c + 1) * 128]
                            for q2 in range(2):
                                nc.tensor.matmul(
                                    out=pq[:, q2 * 512:(q2 + 1) * 512],
                                    lhsT=lhsT,
                                    rhs=xqc[:, q2 * 512:(q2 + 1) * 512],
                                    start=(c == 0), stop=(c == 7))
                        yield do_q
                    def drain_q(pq=pq, qt_t=qt_t):
                        nc.vector.tensor_scalar_add(qt_t, pq, bqc[:, i:i + 1])
                    yield drain_q

                def drive(gen, n=1):
                    if gen is None:
                        return
                    for _ in range(n):
                        for u in gen:
                            u()
                            break
                        else:
                            return

                def finish(gen):
                    if gen is not None:
                        for u in gen:
                            u()

                # prime pairs 0 and 1
                for i in (0, 1):
                    finish(k_chain_units(i))
                    finish(q_chain_units(i))

                for i in range(8):
                    # filler for pair i+2, spread across this pair's chunks
                    filler_k = k_chain_units(i + 2) if i + 2 < 8 else None
                    filler_q = q_chain_units(i + 2) if i + 2 < 8 else None
                    kt_t, qt_t = kt_tiles[i], qt_tiles[i]
                    for h in (2 * i, 2 * i + 1):
                        po = (h % 2) * 64
                        cx = [ctxp.tile([SH, 512], dt.float32, tag="cx",
                                        name=f"cx{h}_{q2}") for q2 in range(2)]
                        for sc in range(16):
                            sp = ps.tile([128, 1024], dt.float32, tag="ps",
                                         name=f"sp{h}_{sc}")
                            lhsT = kt_t[po:po + 64, sc * 128:(sc + 1) * 128]
                            for q2 in range(2):
                                nc.tensor.matmul(
                                    out=sp[:, q2 * 512:(q2 + 1) * 512],
                                    lhsT=lhsT,
                                    rhs=qt_t[po:po + 64, q2 * 512:(q2 + 1) * 512],
                                    start=True, stop=True)
                            e = epool.tile([128, 1024], dt.bfloat16,
                                           tag="e", name=f"e{h}_{sc}")
                            nc.scalar.activation(
                                out=e, in_=sp, func=AF.Exp,
                                bias=mkc[:, sc:sc + 1], scale=1.0 / np.sqrt(DK))
                            vh = v_s[:, sc * H * SH + h * SH:
                                     sc * H * SH + (h + 1) * SH]
                            for q2 in range(2):
                                nc.tensor.matmul(
                                    out=cx[q2], lhsT=vh,
                                    rhs=e[:, q2 * 512:(q2 + 1) * 512],
                                    start=(sc == 0), stop=(sc == 15))
                            # one filler unit per chunk: K during head 2i,
                            # Q during head 2i+1
                            if h == 2 * i:
                                drive(filler_k)
                            else:
                                drive(filler_q if sc >= 8 else filler_k)
                        # drain ctx: copy PSUM->SBUF fast (frees cx slots),
                        # then normalize at leisure
                        st_t = stg.tile([SH, 1024], dt.float32, tag="stg",
                                        name=f"stg{h}")
                        for q2 in range(2):
                            nc.vector.tensor_copy(
                                st_t[:, q2 * 512:(q2 + 1) * 512], cx[q2])
                        rcp = rpool.tile([1, 1024], dt.float32, tag="rcp",
                                         name=f"rcp{h}")
                        nc.vector.reciprocal(out=rcp, in_=st_t[DK:SH, :])
                        rb = rpool.tile([64, 1024], dt.float32, tag="rb",
                                        name=f"rb{h}")
                        nc.gpsimd.partition_broadcast(rb, rcp[0:1, :])
                        i2 = h // 2
                        nc.vector.tensor_mul(
                            out=ctxt_s[po:po + 64, i2 * SQ:(i2 + 1) * SQ],
                            in0=st_t[0:DK, :], in1=rb)
                    finish(filler_k)
                    finish(filler_q)

                # ---- phase 3: output projection -----------------------
                for qc in range(8):
                    pO = ps.tile([128, 1024], dt.float32, tag="ps",
                                 name=f"pO{qc}")
                    for i in range(8):
                        lhsT = ctxt_s[:, i * SQ + qc * 128: i * SQ + (qc + 1) * 128]
                        for do2 in range(2):
                            nc.tensor.matmul(
                                out=pO[:, do2 * 512:(do2 + 1) * 512],
                                lhsT=lhsT,
                                rhs=wo_s[:, i * D + do2 * 512: i * D + (do2 + 1) * 512],
                                start=(i == 0), stop=(i == 7))
                    ot = opool.tile([128, 1024], dt.float32, tag="ot",
                                    name=f"ot{qc}")
                    nc.vector.tensor_add(out=ot, in0=pO, in1=bob)
                    nc.sync.dma_start(
                        out=out[qc * 128:(qc + 1) * 128, :], in_=ot)

    nc.compile()
    return nc


def _make_in_maps(inputs):
    hidden_states = inputs["hidden_states"]
    attention_mask = inputs["attention_mask"]
    wq_b = np.ascontiguousarray(np.asarray(inputs["Wq"]).astype(BF))
    wk_b = np.ascontiguousarray(np.asarray(inputs["Wk"]).astype(BF))
    wv_b = np.ascontiguousarray(np.asarray(inputs["Wv"]).astype(BF))
    wo_b = np.ascontiguousarray(np.asarray(inputs["Wo"]).astype(BF))
    bq_f = np.ascontiguousarray(np.asarray(inputs["bq"]).astype(np.float32))
    bk_f = np.ascontiguousarray(np.asarray(inputs["bk"]).astype(np.float32))
    radd = (np.asarray(inputs["bv"]).astype(np.float32) @
            np.asarray(inputs["Wo"]).astype(np.float32) +
            np.asarray(inputs["bo"]).astype(np.float32))
    radd = np.ascontiguousarray(radd.astype(np.float32))

    in_maps = []
    for c in range(N_CORES):
        b, j = c // 2, c % 2
        xt_b = np.ascontiguousarray(np.asarray(hidden_states[b]).T.astype(BF))
        in_maps.append({
            "xt": xt_b,
            "xqt": np.ascontiguousarray(xt_b[:, j * SQ:(j + 1) * SQ]),
            "wq": wq_b, "wk": wk_b, "wv": wv_b, "wo": wo_b,
            "bq": bq_f, "bk": bk_f, "radd": radd,
            "mk": np.ascontiguousarray(
                np.asarray(attention_mask[b, 0, 0, :]).astype(np.float32) + NEG_C),
        })
    return in_maps


def kernel(hidden_states, attention_mask, Wq, bq, Wk, bk, Wv, bv, Wo, bo):
    from concourse.bass_utils import run_bass_kernel_spmd

    nc = _build()
    in_maps = _make_in_maps(dict(
        hidden_states=hidden_states, attention_mask=attention_mask,
        Wq=Wq, bq=bq, Wk=Wk, bk=bk, Wv=Wv, bv=bv, Wo=Wo, bo=bo))
    res = run_bass_kernel_spmd(nc, in_maps, list(range(N_CORES)))

    full = np.empty((B, S, D), dtype=np.float32)
    for c in range(N_CORES):
        b, j = c // 2, c % 2
        full[b, j * SQ:(j + 1) * SQ, :] = res.results[c]["out"]
    return full
